# revision 1
# baseline (speedup 1.0000x reference)
# ContextQueryAttention (BiDAF-style) Trainium2 Bass/Tile kernel.
#
# Full-input contract: kernel(**inputs) takes the full arrays
#   context [32, 2048, 128] f32, query [32, 128, 128] f32,
#   w [384] f32, query_mask [32, 128] i32
# and returns out [32, 2048, 512] f32.
#
# Sharding: batch B=32 split 4-per-core across 8 NeuronCores (pure data
# parallel, no collectives).
#
# Math (per batch, C=2048, Q=128, D=128):
#   S[c,q] = ctx[c]@w1 + query[q]@w2 + (ctx[c]*w3)@query[q]
#          = alpha[c] + beta[q] + G[c,q]
#   a = softmax_q(S + maskadd);  c2q = a @ query
#   m[c] = max_q(S + maskadd);   b = softmax_c(m); q2c = b @ ctx
#   out = [ctx | c2q | ctx*c2q | ctx*q2c]
#
# Schedule notes (cost-model driven; the kernel is DMA-bound):
#  * DMA floor is ~59.4 us/core (20.3 MB charged at 360 GB/s aggregate);
#    the whole schedule exists to keep DMA_ENGINES saturated end-to-end:
#    - out[:, :, 0:128] = ctx ships as per-batch DRAM->DRAM copies on the
#      gpsimd/SWDGE queue: dependency-free filler that plugs schedule gaps.
#    - ctx loads: 2 up front, then staged one per batch on the SP queue
#      (ctx pool bufs=4 so loads never wait on buffer reuse).
#    - stores ship per 4-tile group (c2q|out3 as one 256-col copy, out4
#      later) so stores stream during the batch instead of at its end.
#  * alpha[c] cancels in softmax_q -> row softmax runs on T = G + beta'
#    (beta' = beta + mask_add) with beta' fused into the ACT exp bias.
#  * E^T = exp(T^T) is stored bf16: the c2q matmul moving operand (query)
#    is bf16 (1 cyc/row vs 4 for fp32).  |S|=O(5) so exp without
#    max-subtraction is safe.
#  * row-max over q comes from a gpsimd partition-reduce on E^T (axis=C)
#    into a [1, C] row, converted to per-tile columns by 16 ap-size-1 PE
#    transposes (~free).  Z[c] comes from ap-size-1 matmuls against a
#    bf16 ones column into a shared multi-writer PSUM bank, with one
#    reciprocal per 4-tile group.
#  * q2c accumulates as a column via u^T += ctx_tile^T @ e_m_col (ap=1
#    matmuls, ~free on PE), then one transpose + broadcast matmul; out4
#    broadcasts q2c over the j dim with a stride-0 AP (broadcast_to).
#  * emission is software-pipelined across batches: batch b's q2c tail is
#    emitted after batch b+1's first ladder group so b+1's in-order
#    engine queues start ~2 us earlier; c2q/normalize for group g is
#    emitted during group g+1's ladder (lag 1).
#  * engine balance per batch (~each < DMA/batch = 14.8 us): ACT ~7
#    (exp, half the ctxT copies, half the normalizes), DVE ~8 (rest of
#    those + out3 + half of out4), Pool ~7 (reduce-max, SWDGE d2d, half
#    of out4), PE ~4.
#  * w and query_mask load as contiguous rows at full DMA rate (their
#    natural column layouts would pay the 7ns/descriptor minimum on
#    4-byte elements) and convert to per-partition columns with ap-size-1
#    PE transposes.
#
#  * late stores (last batch's groups 2-3, second-to-last batch's group
#    3) are split in half: with no dep-free filler left at the tail, the
#    smaller copies overlap their issue-pipe latency with the preceding
#    transfers and compress the final drain.
#
# Result: DMA_ENGINES busy equals total-bytes/360GB/s exactly (58996ns)
# with zero gaps; runtime = 1966ns start (entry barrier + SP issue pipe)
# + 58996ns DMA + 1412ns drain (900ns completion sem + exit barrier).
#
# PSUM (8 banks): big(2) ctx-transpose/S^T staging, acc(3) rotating
# qT/bcol/cq/zb/urow/bc, alpha(1), mz(1) zbank->maxE, u(1) q2c chain.

import numpy as np

C = 2048
Q = 128
D = 128
B_TOTAL = 32
N_CORES = 8
B_LOCAL = B_TOTAL // N_CORES  # 4
N_CT = C // 128  # 16 c-tiles per batch

_compiled = None


def _build():
    import concourse.bacc as bacc
    import concourse.tile as tile
    import concourse.mybir as mybir
    from concourse import masks

    f32 = mybir.dt.float32
    i32 = mybir.dt.int32

    nc = bacc.Bacc(
        "TRN2",
        target_bir_lowering=False,
        debug=False,
        num_devices=N_CORES,
    )

    ctx_d = nc.dram_tensor("context", [B_LOCAL, C, D], f32, kind="ExternalInput").ap()
    qry_d = nc.dram_tensor("query", [B_LOCAL, Q, D], f32, kind="ExternalInput").ap()
    w_d = nc.dram_tensor("w", [3 * D], f32, kind="ExternalInput").ap()
    msk_d = nc.dram_tensor("query_mask", [B_LOCAL, Q], i32, kind="ExternalInput").ap()
    out_d = nc.dram_tensor("out", [B_LOCAL, C, 4 * D], f32, kind="ExternalOutput").ap()

    with tile.TileContext(nc) as tc:
        _kernel_body(tc, out_d, ctx_d, qry_d, w_d, msk_d, mybir, masks)

    nc.compile()
    return nc


def _kernel_body(tc, out_d, ctx_d, qry_d, w_d, msk_d, mybir, masks):
    from contextlib import ExitStack

    nc = tc.nc
    f32 = mybir.dt.float32
    f32r = mybir.dt.float32r
    bf16 = mybir.dt.bfloat16
    i32 = mybir.dt.int32
    AFT = mybir.ActivationFunctionType
    Alu = mybir.AluOpType

    es = ExitStack()
    with es:
        consts = es.enter_context(tc.tile_pool(name="consts", bufs=1))
        sb = es.enter_context(tc.tile_pool(name="sb", bufs=2))
        big = es.enter_context(tc.tile_pool(name="bigsb", bufs=2))
        ps = es.enter_context(tc.tile_pool(name="ps", bufs=1, space="PSUM"))

        # ---- constants ----
        ident = consts.tile([128, 128], f32)
        masks.make_identity(nc, ident[:])
        wrow = consts.tile([1, 3 * D], f32)
        wcols = consts.tile([128, 3], f32)
        ones_col = consts.tile([128, 1], f32)
        nc.vector.memset(ones_col[:], 1.0)
        ones_row = consts.tile([1, 128], bf16)
        nc.vector.memset(ones_row[:], 1.0)
        ones_colb = consts.tile([128, 1], bf16)
        nc.vector.memset(ones_colb[:], 1.0)

        qry_sb = consts.tile([128, B_LOCAL, 128], f32)
        mrow_i = consts.tile([1, B_LOCAL * 128], i32)
        mrow_f = consts.tile([1, B_LOCAL * 128], f32)
        madd4 = consts.tile([128, B_LOCAL], f32)

        ctx_v = ctx_d.rearrange("b (j p) d -> b p j d", p=128)  # [b][128][16][128]
        out_v = out_d.rearrange("b (j p) f -> b p j f", p=128)  # [b][128][16][512]

        # ---- prologue loads (SP queue, all dependency-free) ----
        nc.sync.dma_start(out=qry_sb[:], in_=qry_d.rearrange("b q d -> q b d"))
        ctxs = [
            big.tile([128, N_CT, 128], f32, tag="ctx", bufs=B_LOCAL, name=f"cs{b}")
            for b in range(B_LOCAL)
        ]
        nc.sync.dma_start(out=ctxs[0][:], in_=ctx_v[0])
        nc.sync.dma_start(out=wrow[:], in_=w_d.rearrange("k -> () k"))
        nc.sync.dma_start(out=mrow_i[:], in_=msk_d.rearrange("b q -> () (b q)"))
        nc.sync.dma_start(out=ctxs[1][:], in_=ctx_v[1])

        # w / mask rows -> per-partition columns via ap-size-1 PE transposes
        wcol_ps = ps.tile([128, 3], f32, tag="acc", bufs=3)
        for k in range(3):
            nc.tensor.transpose(
                wcol_ps[:, k : k + 1],
                wrow[:, k * D : (k + 1) * D],
                ones_col[0:1, 0:1],
            )
        nc.vector.tensor_copy(wcols[:], wcol_ps[:])
        w1_col = wcols[:, 0:1]
        w2_col = wcols[:, 1:2]
        w3_col = wcols[:, 2:3]
        # mask add as a row: (maskf - 1) * 1e9, then columns via transposes
        nc.vector.tensor_scalar(
            mrow_f[:], mrow_i[:], 1.0, 1.0e9, op0=Alu.subtract, op1=Alu.mult
        )
        mcol_ps = ps.tile([128, B_LOCAL], f32, tag="acc", bufs=3)
        for k in range(B_LOCAL):
            nc.tensor.transpose(
                mcol_ps[:, k : k + 1],
                mrow_f[:, k * 128 : (k + 1) * 128],
                ones_col[0:1, 0:1],
            )
        nc.vector.tensor_copy(madd4[:], mcol_ps[:])

        pend_tail = None
        for b in range(B_LOCAL):
            cs = ctxs[b]
            at = big.tile([128, N_CT, 384], f32, tag="at", bufs=3)
            if b + 2 < B_LOCAL:
                nc.sync.dma_start(out=ctxs[b + 2][:], in_=ctx_v[b + 2])

            # dep-free DRAM->DRAM filler for this batch (SWDGE: no HWDGE
            # contention, Pool engine pays the descriptor-gen time)
            nc.gpsimd.dma_start(out=out_v[b][:, :, 0:128], in_=ctx_v[b])

            # ---------- A: query prep ----------
            qT_ps = ps.tile([128, 128], f32, tag="acc", bufs=3)
            nc.tensor.transpose(qT_ps[:], qry_sb[:, b, :], ident[:])
            qT = sb.tile([128, 128], f32, tag="qT")
            nc.scalar.copy(qT[:], qT_ps[:])
            qw3T = sb.tile([128, 128], f32r, tag="qw3T")
            nc.vector.tensor_scalar_mul(qw3T[:], qT[:], w3_col)
            bcol_ps = ps.tile([128, 1], f32, tag="acc", bufs=3)
            nc.tensor.matmul(bcol_ps[:], qT[:], w2_col, start=True, stop=True)
            beta_col = sb.tile([128, 1], f32, tag="beta")
            nc.vector.tensor_add(beta_col[:], madd4[:, b : b + 1], bcol_ps[:])
            q_bf = sb.tile([128, 128], bf16, tag="qbf")
            nc.vector.tensor_copy(q_bf[:], qry_sb[:, b, :])

            # ---------- B: group ladder, c2q lagged one group ----------
            ctxT = big.tile([128, C], f32r, tag="ctxT", bufs=2)
            e_t = big.tile([128, C], bf16, tag="et", bufs=2)
            mrow = sb.tile([1, C], f32, tag="mrow")
            alpha_ps = ps.tile([128, N_CT], f32, tag="alpha", bufs=1)
            zbank_ps = ps.tile([128, N_CT], f32, tag="mz", bufs=1)
            rz = sb.tile([128, N_CT], f32, tag="rz")

            def c2q_group(g):
                """cq matmuls + group 1/Z + normalize for group g's tiles."""
                for j in range(4):
                    i = 4 * g + j
                    nc.tensor.matmul(
                        zbank_ps[:, i : i + 1],
                        e_t[:, i * 128 : (i + 1) * 128],
                        ones_colb[:],
                        start=True,
                        stop=True,
                    )
                nc.vector.reciprocal(
                    rz[:, 4 * g : 4 * g + 4], zbank_ps[:, 4 * g : 4 * g + 4]
                )
                for j in range(4):
                    i = 4 * g + j
                    cq_ps = ps.tile([128, 128], f32, tag="acc", bufs=3)
                    nc.tensor.matmul(
                        cq_ps[:],
                        e_t[:, i * 128 : (i + 1) * 128],
                        q_bf[:],
                        start=True,
                        stop=True,
                    )
                    if j % 2 == 0:
                        nc.scalar.activation(
                            out=at[:, i, 0:128],
                            in_=cq_ps[:],
                            func=AFT.Copy,
                            scale=rz[:, i : i + 1],
                        )
                    else:
                        nc.vector.tensor_scalar_mul(
                            at[:, i, 0:128], cq_ps[:], rz[:, i : i + 1]
                        )
                # out3 for this group's 4 tiles (small chunk: keeps the DVE
                # queue free of long convoys), then ship the group's c2q+out3
                lo, hi = 4 * g, 4 * g + 4
                nc.vector.tensor_mul(
                    at[:, lo:hi, 128:256], cs[:, lo:hi, :], at[:, lo:hi, 0:128]
                )
                if (b == B_LOCAL - 1 and g >= 2) or (b == B_LOCAL - 2 and g == 3):
                    nc.sync.dma_start(
                        out=out_v[b][:, lo : lo + 2, 128:384],
                        in_=at[:, lo : lo + 2, 0:256],
                    )
                    nc.sync.dma_start(
                        out=out_v[b][:, lo + 2 : hi, 128:384],
                        in_=at[:, lo + 2 : hi, 0:256],
                    )
                else:
                    nc.sync.dma_start(
                        out=out_v[b][:, lo:hi, 128:384], in_=at[:, lo:hi, 0:256]
                    )

            for g in range(4):
                tr_ps = ps.tile([128, 512], f32, tag="big", bufs=2)
                for j in range(4):
                    nc.tensor.transpose(
                        tr_ps[:, j * 128 : (j + 1) * 128],
                        cs[:, 4 * g + j, :],
                        ident[:],
                    )
                if g % 2 == 0:
                    nc.scalar.copy(ctxT[:, g * 512 : (g + 1) * 512], tr_ps[:])
                else:
                    nc.vector.tensor_copy(ctxT[:, g * 512 : (g + 1) * 512], tr_ps[:])
                for j in range(4):
                    i = 4 * g + j
                    nc.tensor.matmul(
                        alpha_ps[:, i : i + 1],
                        ctxT[:, i * 128 : (i + 1) * 128].bitcast(f32),
                        w1_col,
                        start=True,
                        stop=True,
                    )
                st_ps = ps.tile([128, 512], f32, tag="big", bufs=2)
                nc.tensor.matmul(
                    st_ps[:],
                    qw3T[:],
                    ctxT[:, g * 512 : (g + 1) * 512],
                    start=True,
                    stop=True,
                )
                nc.scalar.activation(
                    out=e_t[:, g * 512 : (g + 1) * 512],
                    in_=st_ps[:],
                    func=AFT.Exp,
                    bias=beta_col[:],
                    scale=1.0,
                )
                # row max / row sum over q: partition reduces on the Pool
                # engine straight from the bf16 E^T block
                nc.gpsimd.reduce_max(
                    out=mrow[:, g * 512 : (g + 1) * 512],
                    in_=e_t[:, g * 512 : (g + 1) * 512],
                    axis=mybir.AxisListType.C,
                )
                if g >= 1:
                    c2q_group(g - 1)
                if g == 0 and pend_tail is not None:
                    pend_tail()
                    pend_tail = None
            c2q_group(3)

            # e_alpha now (frees the single alpha bank before the next batch)
            e_alpha = sb.tile([128, N_CT], f32, tag="ealpha")
            nc.scalar.activation(out=e_alpha[:], in_=alpha_ps[:], func=AFT.Exp)

            def make_tail(b, cs, at, mrow, e_alpha):
                def tail():
                    # ---------- q2c: e_m, u-chain, broadcast, out4 ----------
                    maxE_ps = ps.tile([128, N_CT], f32, tag="mz", bufs=1)
                    for i in range(N_CT):
                        nc.tensor.transpose(
                            maxE_ps[:, i : i + 1],
                            mrow[:, i * 128 : (i + 1) * 128],
                            ones_col[0:1, 0:1],
                        )
                    e_m = sb.tile([128, N_CT], f32, tag="em")
                    nc.vector.tensor_mul(e_m[:], e_alpha[:], maxE_ps[:])
                    zsum = sb.tile([128, 1], f32, tag="zsum")
                    nc.vector.reduce_sum(
                        out=zsum[:], in_=e_m[:], axis=mybir.AxisListType.X
                    )
                    zb_ps = ps.tile([1, 1], f32, tag="acc", bufs=3)
                    nc.tensor.matmul(
                        zb_ps[:], zsum[:], ones_col[:], start=True, stop=True
                    )
                    rzb = sb.tile([1, 1], f32, tag="rzb")
                    nc.vector.reciprocal(rzb[:], zb_ps[:])
                    u_ps = ps.tile([128, 1], f32, tag="u", bufs=1)
                    for i in range(N_CT):
                        nc.tensor.matmul(
                            u_ps[:],
                            cs[:, i, :],
                            e_m[:, i : i + 1],
                            start=(i == 0),
                            stop=(i == N_CT - 1),
                        )
                    q2cT_sb = sb.tile([128, 1], f32, tag="q2ct")
                    nc.vector.tensor_copy(q2cT_sb[:], u_ps[:])
                    urow_ps = ps.tile([1, 128], f32, tag="acc", bufs=3)
                    nc.tensor.transpose(urow_ps[:], q2cT_sb[:], ident[:])
                    q2c_row = sb.tile([1, 128], bf16, tag="q2crow")
                    nc.vector.tensor_scalar_mul(q2c_row[:], urow_ps[:], rzb[:])
                    bc_ps = ps.tile([128, 128], f32, tag="acc", bufs=3)
                    nc.tensor.matmul(
                        bc_ps[:], ones_row[:], q2c_row[:], start=True, stop=True
                    )
                    q2c_sb = sb.tile([128, 128], f32, tag="q2csb")
                    nc.scalar.copy(q2c_sb[:], bc_ps[:])
                    nchunk = 4 if b < B_LOCAL - 1 else 2
                    w = N_CT // nchunk
                    q2c_bc = q2c_sb[:].unsqueeze(1).broadcast_to([128, w, 128])
                    for k in range(nchunk):
                        lo, hi = w * k, w * k + w
                        eng = nc.vector if k % 2 == 0 else nc.gpsimd
                        eng.tensor_mul(
                            at[:, lo:hi, 256:384], cs[:, lo:hi, :], q2c_bc
                        )
                        nc.sync.dma_start(
                            out=out_v[b][:, lo:hi, 384:512],
                            in_=at[:, lo:hi, 256:384],
                        )
                return tail

            pend_tail = make_tail(b, cs, at, mrow, e_alpha)
        pend_tail()


def kernel(**inputs):
    global _compiled
    from concourse.bass_utils import run_bass_kernel_spmd

    context = np.ascontiguousarray(inputs["context"], dtype=np.float32)
    query = np.ascontiguousarray(inputs["query"], dtype=np.float32)
    w = np.ascontiguousarray(inputs["w"], dtype=np.float32)
    qmask = np.ascontiguousarray(inputs["query_mask"], dtype=np.int32)

    if _compiled is None:
        _compiled = _build()
    nc = _compiled

    core_ids = list(range(N_CORES))
    in_maps = []
    for k in core_ids:
        sl = slice(k * B_LOCAL, (k + 1) * B_LOCAL)
        in_maps.append(
            {
                "context": context[sl],
                "query": query[sl],
                "w": w,
                "query_mask": qmask[sl],
            }
        )

    res = run_bass_kernel_spmd(nc, in_maps, core_ids)
    outs = [res.results[k]["out"] for k in range(N_CORES)]
    return np.concatenate(outs, axis=0)

